# revision 19
# baseline (speedup 1.0000x reference)
"""NeighborAttention (B=4, N=4096, K=32, C=128, H=4) on 8 Trainium2 cores.

Data-parallel over the flattened (B*N) node axis; weights replicated.

Mask-sparsity exploitation: ~half the neighbor slots are masked (their E
columns are zero). The host sorts each core's nodes by unmasked-neighbor
count, compacts every node's neighbor list (unmasked first), and chunks
of 512 sorted nodes get a per-chunk neighbor width K' = max count in the
chunk (rounded up to even, shared across cores for SPMD). E[K'/32] ~ 0.58
=> ~42% less DMA/PE/DVE/ACT work. Padded slots behave exactly like masked
ones (E=0 -> s=0 -> e=1, corrected via the host-sent K'-cnt term; uv=0 in
sums and in the max, matching the reference where attn=0 at masked).

Device layout: channel-major [row (4d+h), cols (k, n)], k OUTER within a
chunk so every k-slice is a contiguous 512-col run.

Per 1024-col span (2 k-slices), software-pipelined with a 1-span skew:
    KT = WK' @ ET, VT = WV' @ ET     (PE, fp16 in / fp32 PSUM out)
    ktd = copy(KT)                   (ACT PSUM->SBUF fp16 drain)
    prod = ktd * bcast_k(QT)         (DVE 2x_1p)
    s_rep = Hrep @ prod              (PE) head-summed, d-replicated
    erep = exp(s_rep)                (ACT, PSUM -> SBUF fp16)
    uv   = erep * VT                 (DVE 1x, PSUM fp32 src)
    umax = running max_k uv          (DVE fp16 2x tensor_max)
    usum += I @ uv-slices, z += I @ erep-slices
                                     (PE identity-matmul PSUM accumulation;
                                      k-sums cost PE columns instead of 1x
                                      DVE tensor_reduce passes)
epilogue:
    zc = max(z - (K'-cnt), 1e-4);  out = WOms' @ (usum/zc) + WOmax' @ (umax/zc)

attn sums to exactly 1, so aggr_mean == aggr_sum and the mean/sum W_O
blocks fold together on the host.
"""
import numpy as np
import concourse.bass as bass
import concourse.bacc as bacc
import concourse.mybir as mybir
from concourse import tile
from concourse.bass_utils import run_bass_kernel_spmd

F32 = mybir.dt.float32
F16 = mybir.dt.float16
ALU = mybir.AluOpType
EXP = mybir.ActivationFunctionType.Exp

K = 32
C = 128
H = 4
D = 32
NCORES = 8

NCH = 512                 # nodes per chunk
SPAN = 1024               # cols per inner span (2 k-slices)
HB = 512                  # half-span / matmul N / k-slice width

_NC_CACHE = {}
_PREP_STATE = {}


def build_nc(nloc, kprimes=None):
    if kprimes is None:
        kprimes = _PREP_STATE["kprimes"]
    kprimes = tuple(kprimes)
    key = (nloc, kprimes)
    if key in _NC_CACHE:
        return _NC_CACHE[key]
    assert nloc % NCH == 0
    nchunks = nloc // NCH
    assert len(kprimes) == nchunks
    nsp = [kp // 2 for kp in kprimes]          # spans per chunk
    base = np.concatenate([[0], np.cumsum([kp * NCH for kp in kprimes])])
    totcols = int(base[-1])

    nc = bacc.Bacc()
    et = nc.dram_tensor("et", [C, totcols], F16, kind="ExternalInput")
    xt = nc.dram_tensor("xt", [C, nloc], F16, kind="ExternalInput")
    wqt = nc.dram_tensor("wqt", [C, C], F16, kind="ExternalInput")
    wkt = nc.dram_tensor("wkt", [C, C], F16, kind="ExternalInput")
    wvt = nc.dram_tensor("wvt", [C, C], F16, kind="ExternalInput")
    hrep = nc.dram_tensor("hrep", [C, C], F16, kind="ExternalInput")
    ident = nc.dram_tensor("ident", [C, C], F16, kind="ExternalInput")
    wost = nc.dram_tensor("wost", [C, C], F16, kind="ExternalInput")
    wo3t = nc.dram_tensor("wo3t", [C, C], F16, kind="ExternalInput")
    mcorr = nc.dram_tensor("mcorr", [C, nloc], F16, kind="ExternalInput")
    out = nc.dram_tensor("out", [C, nloc], F32, kind="ExternalOutput")

    with tile.TileContext(nc) as tc:
        with tc.tile_pool(name="wts", bufs=1) as wpool, \
             tc.tile_pool(name="io", bufs=1) as iop, \
             tc.tile_pool(name="etp", bufs=3) as etpool, \
             tc.tile_pool(name="spn", bufs=5) as spool, \
             tc.tile_pool(name="epi", bufs=1) as epip, \
             tc.tile_pool(name="pmain", bufs=1, space="PSUM") as pmain, \
             tc.tile_pool(name="pacc", bufs=1, space="PSUM") as pacc:

            warm = wpool.tile([C, 2], F16, tag="warm")
            nc.vector.memset(warm[:], 0.0)
            warm2 = wpool.tile([C, 2], F16, tag="warm2")
            nc.scalar.activation(warm2[:], warm[:], EXP)

            xt_sb = iop.tile([C, nloc], F16, tag="xt")
            nc.sync.dma_start(xt_sb[:, :HB], xt[:, :HB])

            w_q = wpool.tile([C, C], F16, tag="wq")
            w_k = wpool.tile([C, C], F16, tag="wk")
            w_v = wpool.tile([C, C], F16, tag="wv")
            w_h = wpool.tile([C, C], F16, tag="wh")
            w_i = wpool.tile([C, C], F16, tag="wi")
            w_os = wpool.tile([C, C], F16, tag="wos")
            w_o3 = wpool.tile([C, C], F16, tag="wo3")
            nc.sync.dma_start(w_q[:], wqt[:])
            nc.sync.dma_start(w_k[:], wkt[:])
            nc.sync.dma_start(w_v[:], wvt[:])
            nc.sync.dma_start(w_h[:], hrep[:])
            nc.sync.dma_start(w_i[:], ident[:])
            nc.sync.dma_start(w_os[:], wost[:])
            nc.sync.dma_start(w_o3[:], wo3t[:])
            nc.sync.dma_start(xt_sb[:, HB:], xt[:, HB:])

            et_t = {}

            def load_chunk(ch, pieces=1):
                cols = kprimes[ch] * NCH
                c0 = int(base[ch])
                et_t[ch] = etpool.tile([C, cols], F16, tag="et",
                                       name=f"et{ch}")
                step = cols // pieces
                for p in range(pieces):
                    nc.sync.dma_start(et_t[ch][:, p * step:(p + 1) * step],
                                      et[:, c0 + p * step:c0 + (p + 1) * step])

            # chunk 0 split per span so span 0 can start almost immediately
            load_chunk(0, pieces=kprimes[0] // 2)

            mc_sb = iop.tile([C, nloc], F16, tag="mc")
            nc.sync.dma_start(mc_sb[:], mcorr[:])

            qsb = iop.tile([C, nloc], F16, tag="qsb")
            umax = iop.tile([C, nloc], F16, tag="umax")
            usum_sb = iop.tile([C, nloc], F16, tag="usum")
            out_sb = iop.tile([C, nloc], F32, tag="osb")

            # Q projection: QT = (WQ'/sqrt(d)) @ XT, drained to fp16 SBUF
            for b0 in range(0, nloc, SPAN):
                q_ps = pmain.tile([C, SPAN], F32, tag="sr")
                nc.tensor.matmul(q_ps[:, :HB], w_q[:], xt_sb[:, b0:b0 + HB],
                                 start=True, stop=True)
                nc.tensor.matmul(q_ps[:, HB:], w_q[:], xt_sb[:, b0 + HB:b0 + SPAN],
                                 start=True, stop=True)
                nc.scalar.copy(qsb[:, b0:b0 + SPAN], q_ps[:])

            # Software-pipelined main loop, 1-span skew: stage A(g) =
            # projections + q*k product + head-sum matmul; stage B(g) =
            # exp + e*v + k-reductions. Per-engine emission order matches
            # dependency readiness (engine queues are strict FIFO).
            spans = [(ch, s) for ch in range(nchunks) for s in range(nsp[ch])]
            us_t = {}
            zs_t = {}
            stash = {}

            def stage_a(g):
                ch, s = spans[g]
                if s == 0 and ch + 1 < nchunks and ch + 1 not in et_t:
                    load_chunk(ch + 1, pieces=2)
                et_sb = et_t[ch]
                sl0 = s * SPAN
                kt_ps = pmain.tile([C, SPAN], F32, tag="kt")
                nc.tensor.matmul(kt_ps[:, :HB], w_k[:], et_sb[:, sl0:sl0 + HB],
                                 start=True, stop=True)
                nc.tensor.matmul(kt_ps[:, HB:], w_k[:],
                                 et_sb[:, sl0 + HB:sl0 + SPAN],
                                 start=True, stop=True)
                vt_ps = pmain.tile([C, SPAN], F32, tag="vt")
                nc.tensor.matmul(vt_ps[:, :HB], w_v[:], et_sb[:, sl0:sl0 + HB],
                                 start=True, stop=True)
                nc.tensor.matmul(vt_ps[:, HB:], w_v[:],
                                 et_sb[:, sl0 + HB:sl0 + SPAN],
                                 start=True, stop=True)

                # drain KT to fp16 SBUF on ACT so the q*k multiply runs at
                # the DVE 2x_1p rate instead of the 1x PSUM-fp32 rate
                ktd = spool.tile([C, SPAN], F16, tag="ktd")
                nc.scalar.copy(ktd[:], kt_ps[:])

                n0 = ch * NCH
                qb = qsb[:, n0:n0 + NCH].unsqueeze(1).broadcast_to((C, 2, NCH))
                prod = spool.tile([C, SPAN], F16, tag="pr")
                nc.vector.tensor_mul(
                    prod[:].rearrange("p (k n) -> p k n", n=NCH),
                    ktd[:].rearrange("p (k n) -> p k n", n=NCH),
                    qb)

                sr_ps = pmain.tile([C, SPAN], F32, tag="sr")
                nc.tensor.matmul(sr_ps[:, :HB], w_h[:], prod[:, :HB],
                                 start=True, stop=True)
                nc.tensor.matmul(sr_ps[:, HB:], w_h[:], prod[:, HB:],
                                 start=True, stop=True)

                # every 6th span: also drain VT so that span's e*v multiply
                # runs at 2x; keeps DVE and ACT balanced
                vtd = None
                if g % 6 == 5:
                    vtd = spool.tile([C, SPAN], F16, tag="vtd", bufs=2)
                    nc.scalar.copy(vtd[:], vt_ps[:])
                stash[g] = (sr_ps, vt_ps, vtd)

            def stage_b(g):
                ch, s = spans[g]
                sr_ps, vt_ps, vtd = stash.pop(g)
                n0 = ch * NCH
                last = (s == nsp[ch] - 1)
                if s == 0:
                    us_t[ch] = pacc.tile([C, NCH], F32, tag="us", name="us")
                    zs_t[ch] = pacc.tile([C, NCH], F32, tag="zs", name="zs")
                us_ps, zs_ps = us_t[ch], zs_t[ch]

                erep = spool.tile([C, SPAN], F16, tag="er")
                nc.scalar.activation(erep[:], sr_ps[:], EXP)

                # z += sum_k erep  (identity-matmul accumulation)
                nc.tensor.matmul(zs_ps[:], w_i[:], erep[:, :HB],
                                 start=(s == 0), stop=False)
                nc.tensor.matmul(zs_ps[:], w_i[:], erep[:, HB:],
                                 start=False, stop=False)

                uv = spool.tile([C, SPAN], F16, tag="uv")
                nc.vector.tensor_mul(uv[:], erep[:], vtd[:] if vtd is not None
                                     else vt_ps[:])

                nc.tensor.matmul(us_ps[:], w_i[:], uv[:, :HB],
                                 start=(s == 0), stop=False)
                nc.tensor.matmul(us_ps[:], w_i[:], uv[:, HB:],
                                 start=False, stop=last)

                # running max over k
                um = umax[:, n0:n0 + NCH]
                if s == 0:
                    nc.vector.tensor_max(um, uv[:, :HB], uv[:, HB:])
                else:
                    nc.vector.tensor_max(um, um, uv[:, :HB])
                    nc.vector.tensor_max(um, um, uv[:, HB:])
                if last:
                    # close the z group with the -(K'-cnt) correction column,
                    # then the whole per-chunk epilogue + output, pipelined
                    # in strips of 256 nodes to keep the tail short
                    nc.tensor.matmul(zs_ps[:], w_i[:], mc_sb[:, n0:n0 + NCH],
                                     start=False, stop=True)
                    SB = NCH // 2
                    for st in range(2):
                        a, b = st * SB, (st + 1) * SB
                        nc.scalar.copy(usum_sb[:, n0 + a:n0 + b],
                                       us_ps[:, a:b])
                        zf = epip.tile([C, SB], F32, tag="zf", name="zf",
                                       bufs=3)
                        nc.scalar.copy(zf[:], zs_ps[:, a:b])
                        nc.vector.tensor_scalar_max(zf[:], zf[:], 1e-4)
                        rz32 = epip.tile([C, SB], F32, tag="rz32", name="rz",
                                         bufs=3)
                        nc.vector.reciprocal_approx_fast(rz32[:], zf[:])
                        rz16 = epip.tile([C, SB], F16, tag="rz16", name="rzh",
                                         bufs=3)
                        nc.vector.tensor_copy(rz16[:], rz32[:])
                        wsn = epip.tile([C, SB], F16, tag="wsn", name="wsn",
                                        bufs=3)
                        nc.vector.tensor_mul(
                            wsn[:], usum_sb[:, n0 + a:n0 + b], rz16[:])
                        mxn = epip.tile([C, SB], F16, tag="mxn", name="mxn",
                                        bufs=3)
                        nc.vector.tensor_mul(mxn[:], umax[:, n0 + a:n0 + b],
                                             rz16[:])
                        o_ps = pacc.tile([C, SB], F32, tag="zs", name="ops")
                        nc.tensor.matmul(o_ps[:], w_os[:], wsn[:],
                                         start=True, stop=False)
                        nc.tensor.matmul(o_ps[:], w_o3[:], mxn[:],
                                         start=False, stop=True)
                        nc.scalar.copy(out_sb[:, n0 + a:n0 + b], o_ps[:])
                        nc.sync.dma_start(out[:, n0 + a:n0 + b],
                                          out_sb[:, n0 + a:n0 + b])

            for g in range(len(spans)):
                stage_a(g)
                if g >= 1:
                    stage_b(g - 1)
            stage_b(len(spans) - 1)


    nc.compile()
    _NC_CACHE[key] = nc
    return nc


def _perm_dh(w):
    """[(h*32+d), cin] -> [cin, (4d+h)]"""
    wt = np.asarray(w, dtype=np.float32).reshape(H, D, -1)
    return np.ascontiguousarray(np.transpose(wt, (2, 1, 0)).reshape(-1, H * D))


def prep_inputs(h_X, h_E, mask_attn, W_Q, W_K, W_V, W_O):
    h_X = np.asarray(h_X, dtype=np.float32)
    h_E = np.asarray(h_E, dtype=np.float32)
    mask_attn = np.asarray(mask_attn)
    W_Q = np.asarray(W_Q, dtype=np.float32)
    W_K = np.asarray(W_K, dtype=np.float32)
    W_V = np.asarray(W_V, dtype=np.float32)
    W_O = np.asarray(W_O, dtype=np.float32)

    B, N, Kn, Cin = h_E.shape
    BN = B * N
    nloc = BN // NCORES
    nchunks = nloc // NCH

    maskf = mask_attn.astype(np.float32).reshape(BN, Kn)
    e_m = h_E.reshape(BN, Kn, Cin) * maskf[:, :, None]
    xf = h_X.reshape(BN, -1)
    cnt = maskf.sum(axis=1)

    # compact each node's neighbors: unmasked first (stable), masked (zero)
    # after; then sort each core's nodes by count so low-count nodes share
    # low-K' chunks.
    korder = np.argsort(1.0 - maskf, axis=1, kind="stable")
    e_s = np.take_along_axis(e_m, korder[:, :, None], axis=1)

    perms = []
    percore_kp = np.empty((NCORES, nchunks), np.int64)
    for i in range(NCORES):
        sl = slice(i * nloc, (i + 1) * nloc)
        order = np.argsort(cnt[sl], kind="stable")
        perms.append(order)
        csort = cnt[sl][order]
        for ch in range(nchunks):
            mx = csort[ch * NCH:(ch + 1) * NCH].max()
            percore_kp[i, ch] = max(2, int(np.ceil(mx / 2) * 2))
    kprimes = tuple(int(x) for x in percore_kp.max(axis=0))  # SPMD-shared

    wqt = _perm_dh(W_Q / np.sqrt(D)).astype(np.float16)
    wkt = _perm_dh(W_K).astype(np.float16)
    wvt = _perm_dh(W_V).astype(np.float16)

    idx = np.arange(C)
    hh = idx % H
    hrep = (hh[:, None] == hh[None, :]).astype(np.float16)
    ident = np.eye(C, dtype=np.float16)

    wos = W_O[:, :C] + W_O[:, C:2 * C]
    wo3 = W_O[:, 2 * C:]
    wost = np.ascontiguousarray(
        wos.T.reshape(H, D, C).transpose(1, 0, 2).reshape(C, C)).astype(np.float16)
    wo3t = np.ascontiguousarray(
        wo3.T.reshape(H, D, C).transpose(1, 0, 2).reshape(C, C)).astype(np.float16)

    in_maps = []
    for i in range(NCORES):
        sl = slice(i * nloc, (i + 1) * nloc)
        order = perms[i]
        es_i = e_s[sl][order]                    # [nloc, K, C] sorted nodes
        cnt_i = cnt[sl][order]
        blocks = []
        mcv = np.empty(nloc, np.float32)
        for ch in range(nchunks):
            kp = kprimes[ch]
            blk = es_i[ch * NCH:(ch + 1) * NCH, :kp, :]   # [NCH, kp, C]
            blocks.append(blk.transpose(2, 1, 0).reshape(Cin, kp * NCH))
            mcv[ch * NCH:(ch + 1) * NCH] = cnt_i[ch * NCH:(ch + 1) * NCH] - kp
        etc = np.ascontiguousarray(np.concatenate(blocks, axis=1)).astype(np.float16)
        xtc = np.ascontiguousarray(xf[sl][order].T).astype(np.float16)
        mc = np.ascontiguousarray(
            np.broadcast_to(mcv, (C, nloc))).astype(np.float16)
        in_maps.append({
            "et": etc, "xt": xtc,
            "wqt": wqt, "wkt": wkt, "wvt": wvt, "hrep": hrep, "ident": ident,
            "wost": wost, "wo3t": wo3t, "mcorr": mc,
        })

    _PREP_STATE["kprimes"] = kprimes
    _PREP_STATE["perms"] = perms
    return in_maps, nloc


def assemble_output(results, B, N):
    BN = B * N
    nloc = BN // NCORES
    perms = _PREP_STATE["perms"]
    outf = np.empty((BN, C), np.float32)
    for i, r in enumerate(results):
        outf[i * nloc + perms[i]] = r["out"].T
    return outf.reshape(B, N, C)


def kernel(h_X, h_E, mask_attn, W_Q, W_K, W_V, W_O):
    B, N = np.asarray(h_X).shape[0], np.asarray(h_X).shape[1]
    in_maps, nloc = prep_inputs(h_X, h_E, mask_attn, W_Q, W_K, W_V, W_O)
    nc = build_nc(nloc)
    res = run_bass_kernel_spmd(nc, in_maps, core_ids=list(range(NCORES)))
    return assemble_output(res.results, B, N)
